# revision 23
# baseline (speedup 1.0000x reference)
"""Self-contained Trainium2 Bass kernel for nn_AttentionGate_Wavelet.

kernel(**inputs) takes FULL unsharded inputs (as in reference.setup_inputs())
and returns the FULL output tuple (o1, o2), each [32, 128, 64, 64] float32.

Strategy: pure data parallel over batch (4 images per core, 8 cores), params
replicated; per-conv BN mean/var via two on-device AllReduces, each kicked
off right after the last image's stats for that conv so they overlap the
conv2 tail (and absorb cross-core launch skew as early as possible).  Heavy
matmuls (attention + conv) run in bf16; conv(i) is emission-interleaved with
the attention k-loop of image i+1 so the PE stream stays dense (HAM warm)
while ACT chews the exp stream.  Frames and outputs move as bf16 (host
converts both ways) to halve HBM traffic.  The final BN+lrelu+2x bilinear
upsample per (image, conv): one ACT Prelu (alpha=0.2, per-channel BN
scale/bias) into a persistent zero-padded 34-col tile -> PE identity-matmul
column upsample (3I/I stationaries, col-parity-strided PSUM dst, one PSUM
bank per MM, a tiny fix-MM adds the edge-clamp terms over the zero pads) ->
psum->bf16 copies into a guard-extended row buffer -> flat DVE row upsample
(t3u + two tensor_adds) into row-parity planes -> two strided output DMAs
(128B runs) issued from the gpsimd/sync queues to keep any one queue from
serializing.  Conv outputs y stay in SBUF (bf16); softmax row-sums: ET-side
via ACT accumulators, E-side via DVE bf16 tensor_reduce.
"""
import numpy as np
from contextlib import ExitStack

import concourse.bass as bass
import concourse.tile as tile
from concourse import bacc, mybir
from concourse.ap import AP
from concourse.bass_utils import run_bass_kernel_spmd

F32 = mybir.dt.float32
F32R = mybir.dt.float32r
BF16 = mybir.dt.bfloat16
AF = mybir.ActivationFunctionType
ALU = mybir.AluOpType

N_CORES = 8
BL = 4          # images per core
C = 128


def R(ap):
    return ap.bitcast(F32R)


def _build():
    nc = bacc.Bacc("TRN2", target_bir_lowering=False, debug=False,
                   num_devices=N_CORES)

    f1_d = nc.dram_tensor("frame1", [BL, C, 4096], BF16, kind="ExternalInput")
    f2_d = nc.dram_tensor("frame2", [BL, C, 4096], BF16, kind="ExternalInput")
    wlinT_d = nc.dram_tensor("wlinT", [128, 128], BF16, kind="ExternalInput")
    identb_d = nc.dram_tensor("identb", [128, 128], BF16, kind="ExternalInput")
    ident3b_d = nc.dram_tensor("ident3b", [128, 128], BF16, kind="ExternalInput")
    identf_d = nc.dram_tensor("identf", [128, 128], F32, kind="ExternalInput")
    ones1_d = nc.dram_tensor("ones1", [1, 128], F32, kind="ExternalInput")
    cw_d = nc.dram_tensor("cw", [2, 128, 27, 128], BF16, kind="ExternalInput")
    gspa_d = nc.dram_tensor("gspa", [128, 4], F32, kind="ExternalInput")
    gfre_d = nc.dram_tensor("gfre", [128, 16], BF16, kind="ExternalInput")
    fc13_d = nc.dram_tensor("fc13T", [128, 2, 48], F32, kind="ExternalInput")
    fc24_d = nc.dram_tensor("fc24T", [48, 256], F32, kind="ExternalInput")
    fcf13_d = nc.dram_tensor("fcf13T", [128, 8, 48], F32, kind="ExternalInput")
    fcf24_d = nc.dram_tensor("fcf24T", [48, 1024], F32, kind="ExternalInput")
    b16_d = nc.dram_tensor("bias16", [48, 2], F32, kind="ExternalInput")
    b128_d = nc.dram_tensor("bias128h", [128, 2], F32, kind="ExternalInput")
    bf_d = nc.dram_tensor("biasfh", [128, 8], F32, kind="ExternalInput")
    bng_d = nc.dram_tensor("bng", [128, 2], F32, kind="ExternalInput")
    bnb_d = nc.dram_tensor("bnb", [128, 2], F32, kind="ExternalInput")
    o1_d = nc.dram_tensor("o1", [BL, C, 4096], BF16, kind="ExternalOutput")
    o2_d = nc.dram_tensor("o2", [BL, C, 4096], BF16, kind="ExternalOutput")

    with tile.TileContext(nc) as tc, ExitStack() as ctx:
        cst = ctx.enter_context(tc.tile_pool(name="cst", bufs=1))
        per = ctx.enter_context(tc.tile_pool(name="per", bufs=1))
        sfr = ctx.enter_context(tc.tile_pool(name="sfr", bufs=2))
        sb1 = ctx.enter_context(tc.tile_pool(name="sb1", bufs=1))
        sb2 = ctx.enter_context(tc.tile_pool(name="sb2", bufs=2))
        sb3 = ctx.enter_context(tc.tile_pool(name="sb3", bufs=3))
        sbE = ctx.enter_context(tc.tile_pool(name="sbE", bufs=4))
        fin = ctx.enter_context(tc.tile_pool(name="fin", bufs=2))
        psA = ctx.enter_context(tc.tile_pool(name="psA", bufs=2, space="PSUM"))
        psQ = ctx.enter_context(tc.tile_pool(name="psQ", bufs=1, space="PSUM"))
        psC = ctx.enter_context(tc.tile_pool(name="psC", bufs=2, space="PSUM"))
        dram = ctx.enter_context(tc.tile_pool(name="dram", bufs=1, space="DRAM"))

        # ---- frame DMAs for image 0/1 first (critical path) ----
        frt = {}

        def prep_dma(i):
            tiles = []
            for fd in (f1_d, f2_d):
                for h in range(2):
                    t = sfr.tile([128, 2048], BF16, tag="fr",
                                 name=f"fr_{i}_{len(tiles)}")
                    nc.sync.dma_start(t[:], fd.ap()[i][:, h * 2048:(h + 1) * 2048])
                    tiles.append(t)
            frt[i] = tiles

        prep_dma(0)

        # ---- constants ----
        wlinT = cst.tile([128, 128], BF16)
        identb = cst.tile([128, 128], BF16)
        ident3b = cst.tile([128, 128], BF16)
        identf = cst.tile([128, 128], F32)
        ones1 = cst.tile([1, 128], F32)
        gspa = cst.tile([128, 4], F32)
        gfre = cst.tile([128, 16], BF16)
        fc13 = cst.tile([128, 2, 48], F32)
        fc24 = cst.tile([48, 256], F32)
        fcf13 = cst.tile([128, 8, 48], F32)
        fcf24 = cst.tile([48, 1024], F32)
        b16 = cst.tile([48, 2], F32)
        b128 = cst.tile([128, 2], F32)
        bf = cst.tile([128, 8], F32)
        bng = cst.tile([128, 2], F32)
        bnb = cst.tile([128, 2], F32)
        nc.sync.dma_start(wlinT[:], wlinT_d.ap())
        nc.sync.dma_start(identb[:], identb_d.ap())
        nc.sync.dma_start(ident3b[:], ident3b_d.ap())
        nc.sync.dma_start(R(identf[:]), R(identf_d.ap()))
        nc.sync.dma_start(R(ones1[:]), R(ones1_d.ap()))
        nc.sync.dma_start(R(gspa[:]), R(gspa_d.ap()))
        for t, d in [(fc13, fc13_d), (fc24, fc24_d),
                     (fcf13, fcf13_d), (fcf24, fcf24_d), (b16, b16_d),
                     (b128, b128_d), (bf, bf_d), (bng, bng_d), (bnb, bnb_d)]:
            nc.sync.dma_start(t[:], d.ap())
        nc.sync.dma_start(gfre[:], gfre_d.ap())
        cw1 = cst.tile([128, 27, 128], BF16)
        cw2 = cst.tile([128, 27, 128], BF16)
        nc.sync.dma_start(cw1[:], cw_d.ap()[0])
        nc.sync.dma_start(cw2[:], cw_d.ap()[1])

        prep_dma(1)

        # ---- persistent: double-buffered conv pad tiles (memset once) ----
        pads = []
        for par in range(2):
            cp1 = per.tile([128, 3, 1156], BF16, name=f"cp1_{par}")
            cp2 = per.tile([128, 3, 1156], BF16, name=f"cp2_{par}")
            nc.vector.memset(cp1[:], 0.0)
            nc.vector.memset(cp2[:], 0.0)
            pads.append((cp1, cp2))
        stS = per.tile([128, 16], F32)
        stQ = per.tile([128, 16], F32)
        yst = per.tile([128, 8, 1024], BF16)   # y[j*BL+i] bf16 in SBUF
        # finalize zp double-buffer: cols 0/33 stay zero forever (memset once)
        zps = []
        for par in range(2):
            zpt = per.tile([128, 32, 34], BF16, name=f"zp_{par}")
            nc.vector.memset(zpt[:], 0.0)
            zps.append(zpt)

        # ================= prep: downsample + transposes + G1 =================
        ftiles = {}

        def prep_gps(i):
            """GPS: 64x64 -> 32x32 sum-pool (no /4; folded into scales)."""
            chunks = frt.pop(i)
            f1 = sb3.tile([128, 1024], BF16, tag="f1", name=f"f1_{i}")
            f2 = sb3.tile([128, 1024], BF16, tag="f2", name=f"f2_{i}")
            for ci, fr in enumerate(chunks):
                f = f1 if ci < 2 else f2
                h0 = (ci % 2) * 16
                vc = fr[:].rearrange("p (h a w b) -> p h a w b",
                                     h=16, a=2, w=32, b=2)
                t = sb2.tile([128, 16, 32, 2], BF16, tag="dst",
                             name=f"t_{i}_{ci}")
                nc.gpsimd.tensor_add(t[:], vc[:, :, 0], vc[:, :, 1])
                fv = f[:].rearrange("p (h w) -> p h w", h=32)
                nc.gpsimd.tensor_add(fv[:, h0:h0 + 16, :],
                                     t[:, :, :, 0], t[:, :, :, 1])
            ftiles[i] = (f1, f2)

        def prep_pe(i):
            """PE: per-block transposes of f1/f2 + G1 = W_lin @ f1."""
            f1, f2 = ftiles[i]
            f1T = sb2.tile([128, 8, 128], BF16, tag="fT1", name=f"f1T_{i}")
            f2T = sb2.tile([128, 8, 128], BF16, tag="fT2", name=f"f2T_{i}")
            for (f, fT) in [(f1, f1T), (f2, f2T)]:
                for k in range(8):
                    pt = psA.tile([128, 128], BF16, tag="pa",
                                  name=f"pt_{i}_{fT.name}_{k}")
                    nc.tensor.transpose(pt[:],
                                        f[:, k * 128:(k + 1) * 128], identb[:])
                    nc.vector.tensor_copy(fT[:, k, :], pt[:])
            G1 = sb2.tile([128, 1024], BF16, tag="G1", name=f"G1_{i}")
            for ch in range(2):
                pg = psA.tile([128, 512], F32, tag="pa", name=f"pg_{i}_{ch}")
                nc.tensor.matmul(pg[:], wlinT[:],
                                 f1[:, ch * 512:(ch + 1) * 512],
                                 start=True, stop=True)
                nc.vector.tensor_copy(G1[:, ch * 512:(ch + 1) * 512], pg[:])
            ftiles[i] = (f1, f2, f1T, f2T, G1)

        # ================= k-loop pieces (image i) =================
        kst = {}

        def kloop_begin(i):
            q1p = psQ.tile([128, 1024], F32, tag="q1", name=f"q1_{i}")
            q2p = psQ.tile([128, 1024], F32, tag="q2", name=f"q2_{i}")
            rs2 = sb1.tile([128, 8, 2], F32, tag="rs2", name=f"rs2_{i}")
            rs2b = sb1.tile([128, 8], BF16, tag="rs2b", name=f"rs2b_{i}")
            kst[i] = (q1p, q2p, rs2, rs2b)

        def kloop_A(i, k, side):
            """side 0: ET blocks (pa: f2_k^T stationary x G1); side 1: E."""
            f1, f2, f1T, f2T, G1 = ftiles[i]
            q1p, q2p, rs2, rs2b = kst[i]
            eb = sbE.tile([128, 1024], BF16, tag="eblk",
                          name=f"eb_{i}_{k}_{side}")
            for ch in range(2):
                pa = psA.tile([128, 512], F32, tag="pa",
                              name=f"pa_{i}_{k}_{side}_{ch}")
                if side == 0:
                    nc.tensor.matmul(pa[:], f2[:, k * 128:(k + 1) * 128],
                                     G1[:, ch * 512:(ch + 1) * 512],
                                     start=True, stop=True)
                else:
                    nc.tensor.matmul(pa[:], G1[:, k * 128:(k + 1) * 128],
                                     f2[:, ch * 512:(ch + 1) * 512],
                                     start=True, stop=True)
                if side == 0:
                    nc.scalar.activation(eb[:, ch * 512:(ch + 1) * 512], pa[:],
                                         AF.Exp, scale=0.0625,
                                         accum_out=rs2[:, k, ch:ch + 1])
                else:
                    nc.scalar.activation(eb[:, ch * 512:(ch + 1) * 512], pa[:],
                                         AF.Exp, scale=0.0625)
            if side == 1:
                with nc.allow_low_precision(reason="softmax denom, 1e-2 tol"):
                    nc.vector.tensor_reduce(rs2b[:, k:k + 1], eb[:],
                                            mybir.AxisListType.X, ALU.add)
            return eb

        def kloop_Q(i, k, side, eb):
            f1, f2, f1T, f2T, G1 = ftiles[i]
            q1p, q2p, rs2, rs2b = kst[i]
            qp = q1p if side == 0 else q2p
            fT = f2T if side == 0 else f1T
            for ch in range(2):
                nc.tensor.matmul(qp[:, ch * 512:(ch + 1) * 512],
                                 fT[:, k, :],
                                 eb[:, ch * 512:(ch + 1) * 512],
                                 start=(k == 0), stop=(k == 7))

        # ================= conv event stream (image i) =================
        def conv_events(i):
            """Yields ('mm', fn) and ('stats', j) events; 108 MMs + 2 stats."""
            cp1, cp2 = pads[i % 2]
            for j, (cp, cwt) in enumerate([(cp1, cw1), (cp2, cw2)]):
                pc0 = psC.tile([128, 512], F32, tag="pc", name=f"pc_{i}_{j}_0")
                pc1 = psC.tile([128, 512], F32, tag="pc", name=f"pc_{i}_{j}_1")
                pcs = [pc0, pc1]
                nem = 0
                for g in (2, 0, 1):
                    pv = cp[:, g, :].rearrange("p (r c) -> p r c", r=34)
                    for dy in range(3):
                        for dx in range(3):
                            widx = g * 9 + dy * 3 + dx
                            for ch in range(2):
                                rhs = pv[:, ch * 16 + dy:ch * 16 + dy + 16,
                                         dx:dx + 32]
                                def mk(pc=pcs[ch], w=cwt[:, widx, :], r=rhs,
                                       s=(nem == 0), e=(nem == 26)):
                                    nc.tensor.matmul(pc[:], w, r,
                                                     start=s, stop=e)
                                yield ('mm', mk)
                            nem += 1
                yield ('stats', j, pcs)

        def conv_stats(i, j, pcs):
            for ch in range(2):
                scol = j * 8 + i * 2 + ch
                ydst = yst[:, j * BL + i, ch * 512:(ch + 1) * 512]
                nc.vector.tensor_scalar(ydst, pcs[ch][:], 1.0, 0.0,
                                        ALU.mult, ALU.add,
                                        accum_out=stS[:, scol:scol + 1])
                sq = sb2.tile([128, 512], BF16, tag="sq",
                              name=f"sq_{i}_{j}_{ch}")
                nc.vector.scalar_tensor_tensor(
                    sq[:], ydst, 1.0, ydst, ALU.mult, ALU.mult,
                    accum_out=stQ[:, scol:scol + 1])

        # ================= per-image mid phase =================
        def phase(i):
            f1, f2, f1T, f2T, G1 = ftiles[i]
            q1p, q2p, rs2, rs2b = kst.pop(i)
            cp1, cp2 = pads[i % 2]
            # f (downsampled frame) into conv pad group 2 first: unblocks the
            # frame-group conv taps long before the gating/IWT pads are ready.
            for (f, cp) in [(f1, cp1), (f2, cp2)]:
                pv = cp[:, 2, :].rearrange("p (r c) -> p r c", r=34)
                nc.sync.dma_start(pv[:, 1:33, 1:33],
                                  f[:].rearrange("p (r c) -> p r c", r=32))
            rsd = sb1.tile([128, 16], F32, tag="rsd", name=f"rsd_{i}")
            nc.vector.tensor_add(rsd[:, 0:8],
                                 rs2[:, :, 0], rs2[:, :, 1])
            nc.vector.tensor_copy(rsd[:, 8:16], rs2b[:])

            # ---- normalizers + f_att, pooled accum ----
            f1_att = sb1.tile([128, 1024], F32, tag="att1")
            f2_att = sb1.tile([128, 1024], F32, tag="att2")
            p12h = sb1.tile([128, 4], F32, tag="p12h")
            for (scol, qp, att, pcols) in [(8, q1p, f1_att, (0, 1)),
                                           (0, q2p, f2_att, (2, 3))]:
                # f1_att normalizes by D1 (rsd cols 8..15), f2_att by D2 (0..7)
                rec = sb1.tile([128, 8], F32, tag="rec")
                nc.vector.reciprocal(rec[:], rsd[:, scol:scol + 8])
                ptm = psA.tile([128, 512], F32, tag="pa")
                nc.tensor.transpose(ptm[0:8, 0:128], rec[:], identf[:])
                rT = sb1.tile([8, 128], F32, tag="rT")
                nc.scalar.copy(R(rT[:]), ptm[0:8, 0:128])
                rTf = sb1.tile([1, 1024], F32, tag="rTf")
                nc.sync.dma_start(
                    R(rTf[:].rearrange("p (a b) -> p a b", a=8)), R(rT[:]))
                for half in range(2):
                    pb = psA.tile([128, 512], F32, tag="pa")
                    nc.tensor.matmul(pb[:], R(ones1[:]),
                                     R(rTf[0:1, half * 512:(half + 1) * 512]),
                                     start=True, stop=True)
                    bcs = sb1.tile([128, 512], F32, tag="bcs")
                    nc.vector.tensor_copy(bcs[:], pb[:])
                    nc.vector.scalar_tensor_tensor(
                        R(att[:, half * 512:(half + 1) * 512]),
                        qp[:, half * 512:(half + 1) * 512], 1.0, bcs[:],
                        ALU.mult, ALU.mult,
                        accum_out=p12h[:, pcols[half]:pcols[half] + 1])
            pool12 = sb1.tile([128, 2], F32, tag="pool12")
            nc.vector.tensor_add(pool12[:, 0:1], p12h[:, 0:1], p12h[:, 1:2])
            nc.vector.tensor_add(pool12[:, 1:2], p12h[:, 2:3], p12h[:, 3:4])

            # ---- spatial SE gates (tanh form) ----
            se_cols = sb1.tile([128, 2], F32, tag="secols")
            ph = psC.tile([128, 512], F32, tag="pc")
            for blk in range(2):
                nc.tensor.matmul(ph[0:48, 0:1], fc13[:, blk, :],
                                 pool12[:, blk:blk + 1],
                                 start=(blk == 0), stop=(blk == 1))
            h16 = sb1.tile([48, 1], F32, tag="h16")
            nc.scalar.activation(h16[:], ph[0:48, 0:1], AF.Prelu,
                                 bias=b16[:, 0:1], scale=1.0 / 4096.0,
                                 alpha=0.2)
            for w_, (roff, bcol) in enumerate([(0, 0), (32, 1)]):
                ph2 = psC.tile([128, 512], F32, tag="pc")
                nc.tensor.matmul(ph2[:, 0:1],
                                 fc24[roff:roff + 16, w_ * 128:(w_ + 1) * 128],
                                 h16[roff:roff + 16, :], start=True, stop=True)
                tt = sb1.tile([128, 1], F32, tag="setmp")
                nc.scalar.activation(tt[:], ph2[:, 0:1], AF.Tanh,
                                     bias=b128[:, bcol:bcol + 1], scale=0.5)
                nc.vector.tensor_scalar(se_cols[:, w_:w_ + 1], tt[:],
                                        0.5, 0.5, ALU.mult, ALU.add)

            # ---- spatial gates m1/m2 (tanh) + gated write into conv pads ----
            mrow = sb1.tile([2, 1024], F32, tag="mrow")
            m2r = sb1.tile([1, 1024], F32, tag="m2r")
            for ch in range(2):
                pg = psC.tile([128, 512], F32, tag="pc")
                nc.tensor.matmul(pg[0:2, :], R(gspa[:, 0:2]),
                                 R(f1_att[:, ch * 512:(ch + 1) * 512]),
                                 start=True, stop=False)
                nc.tensor.matmul(pg[0:2, :], R(gspa[:, 2:4]),
                                 R(f2_att[:, ch * 512:(ch + 1) * 512]),
                                 start=False, stop=True)
                nc.scalar.activation(R(mrow[0:2, ch * 512:(ch + 1) * 512]),
                                     pg[0:2, :], AF.Tanh, scale=0.125)
            nc.vector.tensor_scalar(R(mrow[:]), R(mrow[:]), 0.5, 0.5,
                                    ALU.mult, ALU.add)
            nc.sync.dma_start(R(m2r[:]), R(mrow[1:2, :]))
            for (gi, att, qcol, cp) in [(0, f1_att, 1, cp1), (1, f2_att, 0, cp2)]:
                msrc = mrow if gi == 0 else m2r
                cpv = cp[:, 0, :].rearrange("p (r c) -> p r c", r=34)
                for ch in range(2):
                    pb = psA.tile([128, 512], F32, tag="pa")
                    nc.tensor.matmul(pb[:], R(ones1[:]),
                                     R(msrc[0:1, ch * 512:(ch + 1) * 512]),
                                     start=True, stop=True)
                    out = cpv[:, 1 + ch * 16:1 + (ch + 1) * 16, 1:33]
                    nc.vector.scalar_tensor_tensor(
                        out,
                        att[:, ch * 512:(ch + 1) * 512].rearrange(
                            "p (r c) -> p r c", r=16),
                        se_cols[:, qcol:qcol + 1],
                        pb[:].rearrange("p (r c) -> p r c", r=16),
                        ALU.mult, ALU.mult)

            # ---- DWT (no /2) with fused pooled accum ----
            F1 = sb1.tile([128, 4, 256], BF16, tag="fre1")
            F2 = sb1.tile([128, 4, 256], BF16, tag="fre2")
            pf = sb1.tile([128, 8], F32, tag="pf")
            for fi, (att, Ff) in enumerate([(f1_att, F1), (f2_att, F2)]):
                v = att[:].rearrange("p (h a w b) -> p a b h w",
                                     h=16, a=2, w=16, b=2)
                pp = sb1.tile([128, 16, 16], F32, tag="wt0")
                qq = sb1.tile([128, 16, 16], F32, tag="wt1")
                rr = sb1.tile([128, 16, 16], F32, tag="wt2")
                ss = sb1.tile([128, 16, 16], F32, tag="wt3")
                nc.gpsimd.tensor_add(pp[:], v[:, 0, 0], v[:, 1, 0])
                nc.gpsimd.tensor_add(qq[:], v[:, 0, 1], v[:, 1, 1])
                nc.gpsimd.tensor_sub(rr[:], v[:, 1, 0], v[:, 0, 0])
                nc.gpsimd.tensor_sub(ss[:], v[:, 1, 1], v[:, 0, 1])
                for bi, (a0, a1, op) in enumerate([(pp, qq, ALU.add),
                                                   (qq, pp, ALU.subtract),
                                                   (rr, ss, ALU.add),
                                                   (ss, rr, ALU.subtract)]):
                    nc.vector.scalar_tensor_tensor(
                        Ff[:, bi, :].rearrange("p (h w) -> p h w", h=16),
                        a0[:], 1.0, a1[:], ALU.mult, op,
                        accum_out=pf[:, fi * 4 + bi:fi * 4 + bi + 1])

            # ---- freq SE gates (tanh form) ----
            sef_cols = sb1.tile([128, 8], F32, tag="sefcols")
            phf = psC.tile([128, 512], F32, tag="pc")
            for k in range(8):
                nc.tensor.matmul(phf[0:48, 0:1], fcf13[:, k, :],
                                 pf[:, k:k + 1], start=(k == 0), stop=(k == 7))
            h16f = sb1.tile([48, 1], F32, tag="h16f")
            nc.scalar.activation(h16f[:], phf[0:48, 0:1], AF.Prelu,
                                 bias=b16[:, 1:2], scale=1.0 / 2048.0,
                                 alpha=0.2)
            for w_, (roff, ooff) in enumerate([(0, 0), (32, 4)]):
                for blk in range(4):
                    ph2 = psC.tile([128, 512], F32, tag="pc")
                    nc.tensor.matmul(ph2[:, 0:1],
                                     fcf24[roff:roff + 16,
                                           w_ * 512 + blk * 128:
                                           w_ * 512 + (blk + 1) * 128],
                                     h16f[roff:roff + 16, :],
                                     start=True, stop=True)
                    tt = sb1.tile([128, 1], F32, tag="seftmp")
                    nc.scalar.activation(tt[:], ph2[:, 0:1], AF.Tanh,
                                         bias=bf[:, ooff + blk:ooff + blk + 1],
                                         scale=0.5)
                    nc.vector.tensor_scalar(sef_cols[:, ooff + blk:ooff + blk + 1],
                                            tt[:], 0.5, 0.5, ALU.mult, ALU.add)

            # ---- freq gates m1f/m2f (tanh) + in-place gating ----
            mfrow = sb1.tile([2, 256], F32, tag="mfrow")
            m2fr = sb1.tile([1, 256], F32, tag="m2fr")
            pgf = psC.tile([128, 512], F32, tag="pc")
            for k in range(8):
                fsrc = F1 if k < 4 else F2
                nc.tensor.matmul(pgf[0:2, 0:256], gfre[:, 2 * k:2 * k + 2],
                                 fsrc[:, k % 4, :],
                                 start=(k == 0), stop=(k == 7))
            nc.scalar.activation(R(mfrow[0:2, :]), pgf[0:2, 0:256],
                                 AF.Tanh, scale=0.0625)
            nc.vector.tensor_scalar(R(mfrow[:]), R(mfrow[:]), 0.5, 0.5,
                                    ALU.mult, ALU.add)
            nc.sync.dma_start(R(m2fr[:]), R(mfrow[1:2, :]))
            for (gi, Ff, ooff) in [(0, F1, 4), (1, F2, 0)]:
                mfsrc = mfrow if gi == 0 else m2fr
                pbf = psA.tile([128, 512], F32, tag="pa")
                nc.tensor.matmul(pbf[:, 0:256], R(ones1[:]),
                                 R(mfsrc[0:1, 0:256]), start=True, stop=True)
                for blk in range(4):
                    nc.vector.scalar_tensor_tensor(
                        Ff[:, blk, :], Ff[:, blk, :],
                        sef_cols[:, ooff + blk:ooff + blk + 1], pbf[:, 0:256],
                        ALU.mult, ALU.mult)

            # ---- IWT (no /2) straight into conv pad group 1 (GPS) ----
            for (Ff, cp) in [(F1, cp1), (F2, cp2)]:
                uu = sb1.tile([128, 16, 16], F32, tag="wt0")
                vv = sb1.tile([128, 16, 16], F32, tag="wt1")
                ww = sb1.tile([128, 16, 16], F32, tag="wt2")
                zz = sb1.tile([128, 16, 16], F32, tag="wt3")
                x1 = Ff[:, 0, :].rearrange("p (h w) -> p h w", h=16)
                x2 = Ff[:, 1, :].rearrange("p (h w) -> p h w", h=16)
                x3 = Ff[:, 2, :].rearrange("p (h w) -> p h w", h=16)
                x4 = Ff[:, 3, :].rearrange("p (h w) -> p h w", h=16)
                nc.gpsimd.tensor_sub(uu[:], x1, x2)
                nc.gpsimd.tensor_sub(vv[:], x3, x4)
                nc.gpsimd.tensor_add(ww[:], x1, x2)
                nc.gpsimd.tensor_add(zz[:], x3, x4)
                ov = cp[:, 1, :].rearrange("p (r c) -> p r c", r=34)[
                    :, 1:33, 1:33].rearrange("p (h a) (w b) -> p a b h w",
                                             a=2, b=2)
                nc.vector.tensor_sub(ov[:, 0, 0], uu[:], vv[:])
                nc.vector.tensor_add(ov[:, 1, 0], uu[:], vv[:])
                nc.vector.tensor_sub(ov[:, 0, 1], ww[:], zz[:])
                nc.vector.tensor_add(ov[:, 1, 1], ww[:], zz[:])

        # ================= BN allreduce: split start/finish ===============
        NTOT = float(N_CORES * BL * 1024)
        scl = sb1.tile([128, 2], F32, tag="scl")
        bia = sb1.tile([128, 2], F32, tag="bia")
        ccalls = {}

        def bn_start(j):
            ccin = sb1.tile([128, 2], F32, tag=f"ccin{j}")
            nc.vector.tensor_reduce(ccin[:, 0:1],
                                    stS[:, j * 8:(j + 1) * 8],
                                    mybir.AxisListType.X, ALU.add)
            nc.vector.tensor_reduce(ccin[:, 1:2],
                                    stQ[:, j * 8:(j + 1) * 8],
                                    mybir.AxisListType.X, ALU.add)
            cbi = dram.tile([128, 2], F32, name=f"cbi{j}")
            cbo = dram.tile([128, 2], F32, name=f"cbo{j}")
            nc.sync.dma_start(cbi[:], ccin[:])
            nc.gpsimd.collective_compute(
                "AllReduce", ALU.add, replica_groups=[list(range(N_CORES))],
                ins=[cbi[:].opt()], outs=[cbo[:].opt()])
            ccall = sb1.tile([128, 2], F32, tag=f"ccall{j}")
            nc.sync.dma_start(ccall[:], cbo[:])
            ccalls[j] = ccall

        def bn_finish(j):
            ccall = ccalls[j]
            mt = sb1.tile([128, 1], F32, tag=f"mt{j}")
            qt = sb1.tile([128, 1], F32, tag=f"qt{j}")
            nc.vector.tensor_scalar_mul(mt[:], ccall[:, 0:1], 1.0 / NTOT)
            nc.vector.tensor_scalar_mul(qt[:], ccall[:, 1:2], 1.0 / NTOT)
            var = sb1.tile([128, 1], F32, tag=f"var{j}")
            nc.vector.scalar_tensor_tensor(var[:], mt[:], 1.0, mt[:],
                                           ALU.mult, ALU.mult)
            nc.vector.tensor_sub(var[:], qt[:], var[:])
            nc.vector.tensor_scalar_add(var[:], var[:], 1e-5)
            sd = sb1.tile([128, 1], F32, tag=f"sd{j}")
            nc.scalar.activation(sd[:], var[:], AF.Sqrt)
            rstd = sb1.tile([128, 1], F32, tag=f"rstd{j}")
            nc.vector.reciprocal(rstd[:], sd[:])
            sc0 = sb1.tile([128, 1], F32, tag=f"sc0{j}")
            nc.vector.tensor_mul(sc0[:], bng[:, j:j + 1], rstd[:])
            nc.vector.tensor_scalar_mul(scl[:, j:j + 1], sc0[:], 1.0 / 16.0)
            tb = sb1.tile([128, 1], F32, tag=f"tb{j}")
            nc.vector.tensor_mul(tb[:], mt[:], sc0[:])
            nc.vector.tensor_sub(tb[:], bnb[:, j:j + 1], tb[:])
            nc.vector.tensor_scalar_mul(bia[:, j:j + 1], tb[:], 1.0 / 16.0)

        # ================= interleaved block: conv(i) x kloop(i+1) ========
        # Software-pipelined: two A-steps (exp blocks) of kloop(i+1) are
        # emitted BEFORE phase(i) so PE/ACT have dense work covering
        # phase's serial chain; then Q(k) / A(k+2) / conv interleave.
        def block(i):
            ev = conv_events(i) if i >= 0 else iter(())
            nmm = 108 if i >= 0 else 0
            have_k = (i + 1 >= 0) and (i + 1 < BL)

            def emit_conv(n):
                done = 0
                while done < n:
                    try:
                        e = next(ev)
                    except StopIteration:
                        return
                    if e[0] == 'mm':
                        e[1]()
                        done += 1
                    else:
                        conv_stats(i, e[1], e[2])
                        if i == BL - 1:
                            # last image: kick the BN AllReduce for conv j
                            # right after its stats, overlapping the next
                            # conv / finalize work.
                            bn_start(e[1])

            ebs = {}
            if have_k:
                kloop_begin(i + 1)
                for k in (0, 1):
                    ebs[(k, 0)] = kloop_A(i + 1, k, 0)
                    ebs[(k, 1)] = kloop_A(i + 1, k, 1)
            if i >= 0:
                phase(i)
                if i + 2 < BL:
                    prep_dma(i + 2)
                    prep_gps(i + 2)
            if have_k:
                per_seg = max(1, nmm // 12) if nmm else 0
                for k in range(8):
                    kloop_Q(i + 1, k, 0, ebs.pop((k, 0)))
                    kloop_Q(i + 1, k, 1, ebs.pop((k, 1)))
                    if k + 2 < 8:
                        ebs[(k + 2, 0)] = kloop_A(i + 1, k + 2, 0)
                        emit_conv(per_seg)
                        ebs[(k + 2, 1)] = kloop_A(i + 1, k + 2, 1)
                        emit_conv(per_seg)
            # drain remaining conv events
            emit_conv(10 ** 9)
            if i + 2 < BL:
                prep_pe(i + 2)

        # ================= emission =================
        prep_gps(0)
        prep_pe(0)
        prep_gps(1)
        block(-1)          # kloop(0) solo + prep_pe(1)
        for i in range(BL):
            block(i)       # phase(i) runs inside, covered by A-prefetch

        # ================= finalize: BN + lrelu + 2x bilinear upsample ====
        def finalize_ij(i, j, od):
            yv = yst[:, j * BL + i, :]                       # [128,1024] bf16
            # z = prelu(scl*y + bia) into padded 34-col layout (pads zero)
            zp = zps[(j * BL + i) % 2]
            nc.scalar.activation(zp[:, :, 1:33],
                                 yv.rearrange("p (h w) -> p h w", w=32),
                                 AF.Prelu, scale=scl[:, j:j + 1],
                                 bias=bia[:, j:j + 1], alpha=0.2)
            # column upsample on PE: out[h, 2w+b] = 3*z[h,w] + z[h, w-1+2b]
            # (per col-parity: contiguous rhs, parity-strided PSUM dst;
            #  one PSUM bank per MM -- matmul output cannot cross banks;
            #  zero pads + one fix-MM supply the edge clamp terms)
            # ug = u with one clamp-guard row at head and tail
            ug = fin.tile([128, 2176], BF16, tag="ug")
            for q in range(4):
                pw = psQ.tile([128, 512], F32,
                              tag=("q1" if q % 2 == 0 else "q2"),
                              name=f"pw_{i}_{j}_{q}")
                pwq = pw[:].rearrange("p (h w b) -> p h w b", w=32, b=2)
                zq = zp[:, 8 * q:8 * q + 8, :]
                for b in range(2):
                    nc.tensor.matmul(pwq[:, :, :, b], ident3b[:],
                                     zq[:, :, 1:33], start=(b == 0),
                                     stop=False, skip_group_check=True)
                for b in range(2):
                    nc.tensor.matmul(pwq[:, :, :, b], identb[:],
                                     zq[:, :, 2 * b:2 * b + 32],
                                     start=False, stop=False,
                                     skip_group_check=True)
                zqb = zq
                frhs = AP(zqb.tensor, zqb.offset + 1,
                          [list(zqb.ap[0]), [34, 8], [31, 2]])
                pwb = pw[:]
                fdst = AP(pwb.tensor, pwb.offset,
                          [list(pwb.ap[0]), [64, 8], [63, 2]])
                nc.tensor.matmul(fdst, identb[:], frhs,
                                 start=False, stop=True,
                                 skip_group_check=True)
                nc.vector.tensor_copy(ug[:, 64 + 512 * q:64 + 512 * (q + 1)],
                                      pw[:])
            nc.vector.tensor_copy(ug[:, 0:64], ug[:, 64:128])
            nc.vector.tensor_copy(ug[:, 2112:2176], ug[:, 2048:2112])
            # row upsample: pE[h] = 3u[h]+u[h-1], pO[h] = 3u[h]+u[h+1]
            t3u = fin.tile([128, 2048], BF16, tag="t3u")
            nc.vector.tensor_scalar_mul(t3u[:], ug[:, 64:2112], 3.0)
            pE = fin.tile([128, 2048], BF16, tag="pE")
            pO = fin.tile([128, 2048], BF16, tag="pO")
            nc.vector.tensor_add(pE[:], t3u[:], ug[:, 0:2048])
            nc.vector.tensor_add(pO[:], t3u[:], ug[:, 128:2176])
            # store: even rows from pE, odd rows from pO
            ov = od.ap()[i].rearrange("p (h a w) -> p h a w", a=2, w=64)
            nc.gpsimd.dma_start(ov[:, :, 0, :],
                                pE[:].rearrange("p (h w) -> p h w", w=64))
            nc.sync.dma_start(ov[:, :, 1, :],
                              pO[:].rearrange("p (h w) -> p h w", w=64))

        bn_finish(0)
        for i in range(BL):
            finalize_ij(i, 0, o1_d)
        bn_finish(1)
        for i in range(BL):
            finalize_ij(i, 1, o2_d)

    nc.compile()
    return nc


_NC_CACHE = None


def _get_nc():
    global _NC_CACHE
    if _NC_CACHE is None:
        _NC_CACHE = _build()
    return _NC_CACHE


def _prep_weights(inp):
    import ml_dtypes
    bf = ml_dtypes.bfloat16
    g = lambda k: np.ascontiguousarray(np.asarray(inp[k], dtype=np.float32))
    W = {}
    W["wlinT"] = np.ascontiguousarray(g("W_lin").T).astype(bf)
    W["identb"] = np.eye(128, dtype=np.float32).astype(bf)
    W["ident3b"] = (3.0 * np.eye(128, dtype=np.float32)).astype(bf)
    W["identf"] = np.eye(128, dtype=np.float32)
    W["ones1"] = np.ones((1, 128), np.float32)
    cw = np.zeros((2, 128, 27, 128), np.float32)   # (conv, ci, k=(g,dy,dx), o)
    scales = [0.25, 1.0 / 16.0, 0.25]
    for j, name in enumerate(["conv1_w", "conv2_w"]):
        w = g(name)  # [o, 384, 3, 3]
        for gg in range(3):
            blk = w[:, gg * 128:(gg + 1) * 128].transpose(2, 3, 1, 0) * scales[gg]
            cw[j, :, gg * 9:(gg + 1) * 9, :] = blk.reshape(9, 128, 128).transpose(
                1, 0, 2)
    W["cw"] = cw.astype(bf)
    W["gspa"] = np.stack([g("gate1_w")[:128], g("gate2_w")[:128],
                          g("gate1_w")[128:], g("gate2_w")[128:]], axis=1)
    gf = np.zeros((128, 16), np.float32)
    gf[:, 0::2] = g("gate1f_w").reshape(8, 128).T
    gf[:, 1::2] = g("gate2f_w").reshape(8, 128).T
    W["gfre"] = gf.astype(bf)
    fc13 = np.zeros((128, 2, 48), np.float32)
    fc13[:, :, 0:16] = g("fc1_w").T.reshape(2, 128, 16).transpose(1, 0, 2)
    fc13[:, :, 32:48] = g("fc3_w").T.reshape(2, 128, 16).transpose(1, 0, 2)
    W["fc13T"] = fc13
    fc24 = np.zeros((48, 256), np.float32)
    fc24[0:16, 0:128] = g("fc2_w").T
    fc24[32:48, 128:256] = g("fc4_w").T
    W["fc24T"] = fc24
    fcf13 = np.zeros((128, 8, 48), np.float32)
    fcf13[:, :, 0:16] = g("fc1f_w").T.reshape(8, 128, 16).transpose(1, 0, 2)
    fcf13[:, :, 32:48] = g("fc3f_w").T.reshape(8, 128, 16).transpose(1, 0, 2)
    W["fcf13T"] = fcf13
    fcf24 = np.zeros((48, 1024), np.float32)
    fcf24[0:16, 0:512] = g("fc2f_w").T
    fcf24[32:48, 512:1024] = g("fc4f_w").T
    W["fcf24T"] = fcf24
    b16 = np.zeros((48, 2), np.float32)
    b16[0:16, 0] = g("fc1_b"); b16[32:48, 0] = g("fc3_b")
    b16[0:16, 1] = g("fc1f_b"); b16[32:48, 1] = g("fc3f_b")
    W["bias16"] = b16
    # pre-halved for the tanh-sigmoid form: sigma(z+b) = .5 + .5 tanh(.5 z + .5 b)
    W["bias128h"] = 0.5 * np.stack([g("fc2_b"), g("fc4_b")], axis=1)
    W["biasfh"] = 0.5 * np.concatenate([g("fc2f_b").reshape(4, 128).T,
                                        g("fc4f_b").reshape(4, 128).T], axis=1)
    W["bng"] = np.stack([g("bn1_g"), g("bn2_g")], axis=1)
    W["bnb"] = np.stack([g("bn1_b"), g("bn2_b")], axis=1)
    out = {}
    for k, v in W.items():
        out[k] = np.ascontiguousarray(v)
    return out


def run(inputs, trace=False):
    import ml_dtypes
    bf = ml_dtypes.bfloat16
    nc = _get_nc()
    W = _prep_weights(inputs)
    f1 = np.ascontiguousarray(
        np.asarray(inputs["frame1"], np.float32)).reshape(
        32, 128, 4096).astype(bf)
    f2 = np.ascontiguousarray(
        np.asarray(inputs["frame2"], np.float32)).reshape(
        32, 128, 4096).astype(bf)
    in_maps = []
    for c in range(N_CORES):
        m = dict(W)
        m["frame1"] = np.ascontiguousarray(f1[c * BL:(c + 1) * BL])
        m["frame2"] = np.ascontiguousarray(f2[c * BL:(c + 1) * BL])
        in_maps.append(m)
    res = run_bass_kernel_spmd(nc, in_maps, core_ids=list(range(N_CORES)),
                               trace=trace)
    o1 = np.concatenate([np.asarray(res.results[c]["o1"], dtype=np.float32)
                         for c in range(N_CORES)], axis=0)
    o2 = np.concatenate([np.asarray(res.results[c]["o2"], dtype=np.float32)
                         for c in range(N_CORES)], axis=0)
    return (o1.reshape(32, 128, 64, 64), o2.reshape(32, 128, 64, 64)), res


def kernel(**inputs):
    (o1, o2), _ = run(inputs, trace=False)
    return o1, o2


# revision 24
# speedup vs baseline: 1.0165x; 1.0165x over previous
"""Self-contained Trainium2 Bass kernel for nn_AttentionGate_Wavelet.

kernel(**inputs) takes FULL unsharded inputs (as in reference.setup_inputs())
and returns the FULL output tuple (o1, o2), each [32, 128, 64, 64] float32.

Strategy: pure data parallel over batch (4 images per core, 8 cores), params
replicated; per-conv BN mean/var via two on-device AllReduces, each kicked
off right after the last image's stats for that conv so they overlap the
conv2 tail (and absorb cross-core launch skew as early as possible).  Heavy
matmuls (attention + conv) run in bf16; conv(i) is emission-interleaved with
the attention k-loop of image i+1 so the PE stream stays dense (HAM warm)
while ACT chews the exp stream.  Frames and outputs move as bf16 (host
converts both ways) to halve HBM traffic.  The final BN+lrelu+2x bilinear
upsample per (image, conv): one ACT Prelu (alpha=0.2, per-channel BN
scale/bias) into a persistent zero-padded 34-col tile -> PE identity-matmul
column upsample (3I/I stationaries, col-parity-strided PSUM dst, one PSUM
bank per MM, a tiny fix-MM adds the edge-clamp terms over the zero pads) ->
psum->bf16 copies into a guard-extended row buffer -> flat DVE row upsample
(t3u + two tensor_adds) into row-parity planes -> two strided output DMAs
(128B runs) issued from the gpsimd/sync queues to keep any one queue from
serializing.  Conv outputs y stay in SBUF (bf16); softmax row-sums: ET-side
via ACT accumulators, E-side via DVE bf16 tensor_reduce.
"""
import numpy as np
from contextlib import ExitStack

import concourse.bass as bass
import concourse.tile as tile
from concourse import bacc, mybir
from concourse.ap import AP
from concourse.bass_utils import run_bass_kernel_spmd

F32 = mybir.dt.float32
F32R = mybir.dt.float32r
BF16 = mybir.dt.bfloat16
AF = mybir.ActivationFunctionType
ALU = mybir.AluOpType

N_CORES = 8
BL = 4          # images per core
C = 128


def R(ap):
    return ap.bitcast(F32R)


def _build():
    nc = bacc.Bacc("TRN2", target_bir_lowering=False, debug=False,
                   num_devices=N_CORES)

    f1_d = nc.dram_tensor("frame1", [BL, C, 4096], BF16, kind="ExternalInput")
    f2_d = nc.dram_tensor("frame2", [BL, C, 4096], BF16, kind="ExternalInput")
    wlinT_d = nc.dram_tensor("wlinT", [128, 128], BF16, kind="ExternalInput")
    identb_d = nc.dram_tensor("identb", [128, 128], BF16, kind="ExternalInput")
    ident3b_d = nc.dram_tensor("ident3b", [128, 128], BF16, kind="ExternalInput")
    identf_d = nc.dram_tensor("identf", [128, 128], F32, kind="ExternalInput")
    ones1_d = nc.dram_tensor("ones1", [1, 128], F32, kind="ExternalInput")
    cw_d = nc.dram_tensor("cw", [2, 128, 27, 128], BF16, kind="ExternalInput")
    gspa_d = nc.dram_tensor("gspa", [128, 4], F32, kind="ExternalInput")
    gfre_d = nc.dram_tensor("gfre", [128, 16], BF16, kind="ExternalInput")
    fc13_d = nc.dram_tensor("fc13T", [128, 2, 48], F32, kind="ExternalInput")
    fc24_d = nc.dram_tensor("fc24T", [48, 256], F32, kind="ExternalInput")
    fcf13_d = nc.dram_tensor("fcf13T", [128, 8, 48], F32, kind="ExternalInput")
    fcf24_d = nc.dram_tensor("fcf24T", [48, 1024], F32, kind="ExternalInput")
    b16_d = nc.dram_tensor("bias16", [48, 2], F32, kind="ExternalInput")
    b128_d = nc.dram_tensor("bias128h", [128, 2], F32, kind="ExternalInput")
    bf_d = nc.dram_tensor("biasfh", [128, 8], F32, kind="ExternalInput")
    bng_d = nc.dram_tensor("bng", [128, 2], F32, kind="ExternalInput")
    bnb_d = nc.dram_tensor("bnb", [128, 2], F32, kind="ExternalInput")
    o1_d = nc.dram_tensor("o1", [BL, C, 4096], BF16, kind="ExternalOutput")
    o2_d = nc.dram_tensor("o2", [BL, C, 4096], BF16, kind="ExternalOutput")

    with tile.TileContext(nc) as tc, ExitStack() as ctx:
        cst = ctx.enter_context(tc.tile_pool(name="cst", bufs=1))
        per = ctx.enter_context(tc.tile_pool(name="per", bufs=1))
        sfr = ctx.enter_context(tc.tile_pool(name="sfr", bufs=2))
        sb1 = ctx.enter_context(tc.tile_pool(name="sb1", bufs=1))
        sb2 = ctx.enter_context(tc.tile_pool(name="sb2", bufs=2))
        sb3 = ctx.enter_context(tc.tile_pool(name="sb3", bufs=3))
        sbE = ctx.enter_context(tc.tile_pool(name="sbE", bufs=4))
        fin = ctx.enter_context(tc.tile_pool(name="fin", bufs=2))
        psA = ctx.enter_context(tc.tile_pool(name="psA", bufs=2, space="PSUM"))
        psQ = ctx.enter_context(tc.tile_pool(name="psQ", bufs=1, space="PSUM"))
        psC = ctx.enter_context(tc.tile_pool(name="psC", bufs=2, space="PSUM"))
        dram = ctx.enter_context(tc.tile_pool(name="dram", bufs=1, space="DRAM"))

        # ---- frame DMAs for image 0/1 first (critical path) ----
        frt = {}

        def prep_dma(i):
            tiles = []
            for fd in (f1_d, f2_d):
                for h in range(2):
                    t = sfr.tile([128, 2048], BF16, tag="fr",
                                 name=f"fr_{i}_{len(tiles)}")
                    nc.sync.dma_start(t[:], fd.ap()[i][:, h * 2048:(h + 1) * 2048])
                    tiles.append(t)
            frt[i] = tiles

        prep_dma(0)

        # ---- constants ----
        wlinT = cst.tile([128, 128], BF16)
        identb = cst.tile([128, 128], BF16)
        ident3b = cst.tile([128, 128], BF16)
        identf = cst.tile([128, 128], F32)
        ones1 = cst.tile([1, 128], F32)
        gspa = cst.tile([128, 4], F32)
        gfre = cst.tile([128, 16], BF16)
        fc13 = cst.tile([128, 2, 48], F32)
        fc24 = cst.tile([48, 256], F32)
        fcf13 = cst.tile([128, 8, 48], F32)
        fcf24 = cst.tile([48, 1024], F32)
        b16 = cst.tile([48, 2], F32)
        b128 = cst.tile([128, 2], F32)
        bf = cst.tile([128, 8], F32)
        bng = cst.tile([128, 2], F32)
        bnb = cst.tile([128, 2], F32)
        nc.sync.dma_start(wlinT[:], wlinT_d.ap())
        nc.sync.dma_start(identb[:], identb_d.ap())
        nc.sync.dma_start(ident3b[:], ident3b_d.ap())
        nc.sync.dma_start(R(identf[:]), R(identf_d.ap()))
        nc.sync.dma_start(R(ones1[:]), R(ones1_d.ap()))
        nc.sync.dma_start(R(gspa[:]), R(gspa_d.ap()))
        for t, d in [(fc13, fc13_d), (fc24, fc24_d),
                     (fcf13, fcf13_d), (fcf24, fcf24_d), (b16, b16_d),
                     (b128, b128_d), (bf, bf_d), (bng, bng_d), (bnb, bnb_d)]:
            nc.sync.dma_start(t[:], d.ap())
        nc.sync.dma_start(gfre[:], gfre_d.ap())
        cw1 = cst.tile([128, 27, 128], BF16)
        cw2 = cst.tile([128, 27, 128], BF16)
        nc.sync.dma_start(cw1[:], cw_d.ap()[0])
        nc.sync.dma_start(cw2[:], cw_d.ap()[1])

        prep_dma(1)

        # ---- persistent: double-buffered conv pad tiles (memset once) ----
        pads = []
        for par in range(2):
            cp1 = per.tile([128, 3, 1156], BF16, name=f"cp1_{par}")
            cp2 = per.tile([128, 3, 1156], BF16, name=f"cp2_{par}")
            nc.vector.memset(cp1[:], 0.0)
            nc.vector.memset(cp2[:], 0.0)
            pads.append((cp1, cp2))
        stS = per.tile([128, 16], F32)
        stQ = per.tile([128, 16], F32)
        yst = per.tile([128, 8, 1024], BF16)   # y[j*BL+i] bf16 in SBUF
        # finalize zp double-buffer: cols 0/33 stay zero forever (memset once)
        zps = []
        for par in range(2):
            zpt = per.tile([128, 32, 34], BF16, name=f"zp_{par}")
            nc.vector.memset(zpt[:], 0.0)
            zps.append(zpt)

        # ================= prep: downsample + transposes + G1 =================
        ftiles = {}

        def prep_gps(i):
            """GPS: 64x64 -> 32x32 sum-pool (no /4; folded into scales)."""
            chunks = frt.pop(i)
            f1 = sb3.tile([128, 1024], BF16, tag="f1", name=f"f1_{i}")
            f2 = sb3.tile([128, 1024], BF16, tag="f2", name=f"f2_{i}")
            for ci, fr in enumerate(chunks):
                f = f1 if ci < 2 else f2
                h0 = (ci % 2) * 16
                vc = fr[:].rearrange("p (h a w b) -> p h a w b",
                                     h=16, a=2, w=32, b=2)
                t = sb2.tile([128, 16, 32, 2], BF16, tag="dst",
                             name=f"t_{i}_{ci}")
                nc.gpsimd.tensor_add(t[:], vc[:, :, 0], vc[:, :, 1])
                fv = f[:].rearrange("p (h w) -> p h w", h=32)
                nc.gpsimd.tensor_add(fv[:, h0:h0 + 16, :],
                                     t[:, :, :, 0], t[:, :, :, 1])
            ftiles[i] = (f1, f2)

        def prep_pe(i):
            """PE: per-block transposes of f1/f2 + G1 = W_lin @ f1."""
            f1, f2 = ftiles[i]
            f1T = sb2.tile([128, 8, 128], BF16, tag="fT1", name=f"f1T_{i}")
            f2T = sb2.tile([128, 8, 128], BF16, tag="fT2", name=f"f2T_{i}")
            for (f, fT) in [(f1, f1T), (f2, f2T)]:
                for k in range(8):
                    pt = psA.tile([128, 128], BF16, tag="pa",
                                  name=f"pt_{i}_{fT.name}_{k}")
                    nc.tensor.transpose(pt[:],
                                        f[:, k * 128:(k + 1) * 128], identb[:])
                    nc.vector.tensor_copy(fT[:, k, :], pt[:])
            G1 = sb2.tile([128, 1024], BF16, tag="G1", name=f"G1_{i}")
            for ch in range(2):
                pg = psA.tile([128, 512], F32, tag="pa", name=f"pg_{i}_{ch}")
                nc.tensor.matmul(pg[:], wlinT[:],
                                 f1[:, ch * 512:(ch + 1) * 512],
                                 start=True, stop=True)
                nc.vector.tensor_copy(G1[:, ch * 512:(ch + 1) * 512], pg[:])
            ftiles[i] = (f1, f2, f1T, f2T, G1)

        # ================= k-loop pieces (image i) =================
        kst = {}

        def kloop_begin(i):
            q1p = psQ.tile([128, 1024], F32, tag="q1", name=f"q1_{i}")
            q2p = psQ.tile([128, 1024], F32, tag="q2", name=f"q2_{i}")
            rs2 = sb1.tile([128, 8, 2], F32, tag="rs2", name=f"rs2_{i}")
            rs2b = sb1.tile([128, 8], BF16, tag="rs2b", name=f"rs2b_{i}")
            kst[i] = (q1p, q2p, rs2, rs2b)

        def kloop_A(i, k, side):
            """side 0: ET blocks (pa: f2_k^T stationary x G1); side 1: E."""
            f1, f2, f1T, f2T, G1 = ftiles[i]
            q1p, q2p, rs2, rs2b = kst[i]
            eb = sbE.tile([128, 1024], BF16, tag="eblk",
                          name=f"eb_{i}_{k}_{side}")
            for ch in range(2):
                pa = psA.tile([128, 512], F32, tag="pa",
                              name=f"pa_{i}_{k}_{side}_{ch}")
                if side == 0:
                    nc.tensor.matmul(pa[:], f2[:, k * 128:(k + 1) * 128],
                                     G1[:, ch * 512:(ch + 1) * 512],
                                     start=True, stop=True)
                else:
                    nc.tensor.matmul(pa[:], G1[:, k * 128:(k + 1) * 128],
                                     f2[:, ch * 512:(ch + 1) * 512],
                                     start=True, stop=True)
                if side == 0:
                    nc.scalar.activation(eb[:, ch * 512:(ch + 1) * 512], pa[:],
                                         AF.Exp, scale=0.0625,
                                         accum_out=rs2[:, k, ch:ch + 1])
                else:
                    nc.scalar.activation(eb[:, ch * 512:(ch + 1) * 512], pa[:],
                                         AF.Exp, scale=0.0625)
            if side == 1:
                with nc.allow_low_precision(reason="softmax denom, 1e-2 tol"):
                    nc.vector.tensor_reduce(rs2b[:, k:k + 1], eb[:],
                                            mybir.AxisListType.X, ALU.add)
            return eb

        def kloop_Q(i, k, side, eb):
            f1, f2, f1T, f2T, G1 = ftiles[i]
            q1p, q2p, rs2, rs2b = kst[i]
            qp = q1p if side == 0 else q2p
            fT = f2T if side == 0 else f1T
            for ch in range(2):
                nc.tensor.matmul(qp[:, ch * 512:(ch + 1) * 512],
                                 fT[:, k, :],
                                 eb[:, ch * 512:(ch + 1) * 512],
                                 start=(k == 0), stop=(k == 7))

        # ================= conv event stream (image i) =================
        def conv_events(i):
            """Yields ('mm', fn) and ('stats', j) events; 108 MMs + 2 stats."""
            cp1, cp2 = pads[i % 2]
            for j, (cp, cwt) in enumerate([(cp1, cw1), (cp2, cw2)]):
                pc0 = psC.tile([128, 512], F32, tag="pc", name=f"pc_{i}_{j}_0")
                pc1 = psC.tile([128, 512], F32, tag="pc", name=f"pc_{i}_{j}_1")
                pcs = [pc0, pc1]
                nem = 0
                for g in (2, 0, 1):
                    pv = cp[:, g, :].rearrange("p (r c) -> p r c", r=34)
                    for dy in range(3):
                        for dx in range(3):
                            widx = g * 9 + dy * 3 + dx
                            for ch in range(2):
                                rhs = pv[:, ch * 16 + dy:ch * 16 + dy + 16,
                                         dx:dx + 32]
                                def mk(pc=pcs[ch], w=cwt[:, widx, :], r=rhs,
                                       s=(nem == 0), e=(nem == 26)):
                                    nc.tensor.matmul(pc[:], w, r,
                                                     start=s, stop=e)
                                yield ('mm', mk)
                            nem += 1
                yield ('stats', j, pcs)

        def conv_stats(i, j, pcs):
            for ch in range(2):
                scol = j * 8 + i * 2 + ch
                ydst = yst[:, j * BL + i, ch * 512:(ch + 1) * 512]
                nc.vector.tensor_scalar(ydst, pcs[ch][:], 1.0, 0.0,
                                        ALU.mult, ALU.add,
                                        accum_out=stS[:, scol:scol + 1])
                sq = sb2.tile([128, 512], BF16, tag="sq",
                              name=f"sq_{i}_{j}_{ch}")
                nc.vector.scalar_tensor_tensor(
                    sq[:], ydst, 1.0, ydst, ALU.mult, ALU.mult,
                    accum_out=stQ[:, scol:scol + 1])

        # ================= per-image mid phase =================
        def phase(i):
            f1, f2, f1T, f2T, G1 = ftiles[i]
            q1p, q2p, rs2, rs2b = kst.pop(i)
            cp1, cp2 = pads[i % 2]
            # f (downsampled frame) into conv pad group 2 first: unblocks the
            # frame-group conv taps long before the gating/IWT pads are ready.
            for (f, cp) in [(f1, cp1), (f2, cp2)]:
                pv = cp[:, 2, :].rearrange("p (r c) -> p r c", r=34)
                nc.scalar.copy(pv[:, 1:33, 1:33],
                               f[:].rearrange("p (r c) -> p r c", r=32))
            rsd = sb1.tile([128, 16], F32, tag="rsd", name=f"rsd_{i}")
            nc.vector.tensor_add(rsd[:, 0:8],
                                 rs2[:, :, 0], rs2[:, :, 1])
            nc.vector.tensor_copy(rsd[:, 8:16], rs2b[:])

            # ---- normalizers + f_att, pooled accum ----
            f1_att = sb1.tile([128, 1024], F32, tag="att1")
            f2_att = sb1.tile([128, 1024], F32, tag="att2")
            p12h = sb1.tile([128, 4], F32, tag="p12h")
            for (scol, qp, att, pcols) in [(8, q1p, f1_att, (0, 1)),
                                           (0, q2p, f2_att, (2, 3))]:
                # f1_att normalizes by D1 (rsd cols 8..15), f2_att by D2 (0..7)
                rec = sb1.tile([128, 8], F32, tag="rec")
                nc.vector.reciprocal(rec[:], rsd[:, scol:scol + 8])
                ptm = psA.tile([128, 512], F32, tag="pa")
                nc.tensor.transpose(ptm[0:8, 0:128], rec[:], identf[:])
                rT = sb1.tile([8, 128], F32, tag="rT")
                nc.scalar.copy(R(rT[:]), ptm[0:8, 0:128])
                rTf = sb1.tile([1, 1024], F32, tag="rTf")
                nc.sync.dma_start(
                    R(rTf[:].rearrange("p (a b) -> p a b", a=8)), R(rT[:]))
                for half in range(2):
                    pb = psA.tile([128, 512], F32, tag="pa")
                    nc.tensor.matmul(pb[:], R(ones1[:]),
                                     R(rTf[0:1, half * 512:(half + 1) * 512]),
                                     start=True, stop=True)
                    bcs = sb1.tile([128, 512], F32, tag="bcs")
                    if half == 0:
                        nc.scalar.copy(bcs[:], pb[:])
                    else:
                        nc.vector.tensor_copy(bcs[:], pb[:])
                    nc.vector.scalar_tensor_tensor(
                        R(att[:, half * 512:(half + 1) * 512]),
                        qp[:, half * 512:(half + 1) * 512], 1.0, bcs[:],
                        ALU.mult, ALU.mult,
                        accum_out=p12h[:, pcols[half]:pcols[half] + 1])
            pool12 = sb1.tile([128, 2], F32, tag="pool12")
            nc.vector.tensor_add(pool12[:, 0:1], p12h[:, 0:1], p12h[:, 1:2])
            nc.vector.tensor_add(pool12[:, 1:2], p12h[:, 2:3], p12h[:, 3:4])

            # ---- spatial SE gates (tanh form) ----
            se_cols = sb1.tile([128, 2], F32, tag="secols")
            ph = psC.tile([128, 512], F32, tag="pc")
            for blk in range(2):
                nc.tensor.matmul(ph[0:48, 0:1], fc13[:, blk, :],
                                 pool12[:, blk:blk + 1],
                                 start=(blk == 0), stop=(blk == 1))
            h16 = sb1.tile([48, 1], F32, tag="h16")
            nc.scalar.activation(h16[:], ph[0:48, 0:1], AF.Prelu,
                                 bias=b16[:, 0:1], scale=1.0 / 4096.0,
                                 alpha=0.2)
            for w_, (roff, bcol) in enumerate([(0, 0), (32, 1)]):
                ph2 = psC.tile([128, 512], F32, tag="pc")
                nc.tensor.matmul(ph2[:, 0:1],
                                 fc24[roff:roff + 16, w_ * 128:(w_ + 1) * 128],
                                 h16[roff:roff + 16, :], start=True, stop=True)
                tt = sb1.tile([128, 1], F32, tag="setmp")
                nc.scalar.activation(tt[:], ph2[:, 0:1], AF.Tanh,
                                     bias=b128[:, bcol:bcol + 1], scale=0.5)
                nc.vector.tensor_scalar(se_cols[:, w_:w_ + 1], tt[:],
                                        0.5, 0.5, ALU.mult, ALU.add)

            # ---- spatial gates m1/m2 (tanh) + gated write into conv pads ----
            mrow = sb1.tile([2, 1024], F32, tag="mrow")
            m2r = sb1.tile([1, 1024], F32, tag="m2r")
            for ch in range(2):
                pg = psC.tile([128, 512], F32, tag="pc")
                nc.tensor.matmul(pg[0:2, :], R(gspa[:, 0:2]),
                                 R(f1_att[:, ch * 512:(ch + 1) * 512]),
                                 start=True, stop=False)
                nc.tensor.matmul(pg[0:2, :], R(gspa[:, 2:4]),
                                 R(f2_att[:, ch * 512:(ch + 1) * 512]),
                                 start=False, stop=True)
                nc.scalar.activation(R(mrow[0:2, ch * 512:(ch + 1) * 512]),
                                     pg[0:2, :], AF.Tanh, scale=0.125)
            nc.vector.tensor_scalar(R(mrow[:]), R(mrow[:]), 0.5, 0.5,
                                    ALU.mult, ALU.add)
            nc.sync.dma_start(R(m2r[:]), R(mrow[1:2, :]))
            for (gi, att, qcol, cp) in [(0, f1_att, 1, cp1), (1, f2_att, 0, cp2)]:
                msrc = mrow if gi == 0 else m2r
                cpv = cp[:, 0, :].rearrange("p (r c) -> p r c", r=34)
                for ch in range(2):
                    pb = psA.tile([128, 512], F32, tag="pa")
                    nc.tensor.matmul(pb[:], R(ones1[:]),
                                     R(msrc[0:1, ch * 512:(ch + 1) * 512]),
                                     start=True, stop=True)
                    out = cpv[:, 1 + ch * 16:1 + (ch + 1) * 16, 1:33]
                    nc.vector.scalar_tensor_tensor(
                        out,
                        att[:, ch * 512:(ch + 1) * 512].rearrange(
                            "p (r c) -> p r c", r=16),
                        se_cols[:, qcol:qcol + 1],
                        pb[:].rearrange("p (r c) -> p r c", r=16),
                        ALU.mult, ALU.mult)

            # ---- DWT (no /2) with fused pooled accum ----
            F1 = sb1.tile([128, 4, 256], BF16, tag="fre1")
            F2 = sb1.tile([128, 4, 256], BF16, tag="fre2")
            pf = sb1.tile([128, 8], F32, tag="pf")
            for fi, (att, Ff) in enumerate([(f1_att, F1), (f2_att, F2)]):
                v = att[:].rearrange("p (h a w b) -> p a b h w",
                                     h=16, a=2, w=16, b=2)
                pp = sb1.tile([128, 16, 16], F32, tag="wt0")
                qq = sb1.tile([128, 16, 16], F32, tag="wt1")
                rr = sb1.tile([128, 16, 16], F32, tag="wt2")
                ss = sb1.tile([128, 16, 16], F32, tag="wt3")
                nc.gpsimd.tensor_add(pp[:], v[:, 0, 0], v[:, 1, 0])
                nc.gpsimd.tensor_add(qq[:], v[:, 0, 1], v[:, 1, 1])
                nc.gpsimd.tensor_sub(rr[:], v[:, 1, 0], v[:, 0, 0])
                nc.gpsimd.tensor_sub(ss[:], v[:, 1, 1], v[:, 0, 1])
                for bi, (a0, a1, op) in enumerate([(pp, qq, ALU.add),
                                                   (qq, pp, ALU.subtract),
                                                   (rr, ss, ALU.add),
                                                   (ss, rr, ALU.subtract)]):
                    nc.vector.scalar_tensor_tensor(
                        Ff[:, bi, :].rearrange("p (h w) -> p h w", h=16),
                        a0[:], 1.0, a1[:], ALU.mult, op,
                        accum_out=pf[:, fi * 4 + bi:fi * 4 + bi + 1])

            # ---- freq SE gates (tanh form) ----
            sef_cols = sb1.tile([128, 8], F32, tag="sefcols")
            phf = psC.tile([128, 512], F32, tag="pc")
            for k in range(8):
                nc.tensor.matmul(phf[0:48, 0:1], fcf13[:, k, :],
                                 pf[:, k:k + 1], start=(k == 0), stop=(k == 7))
            h16f = sb1.tile([48, 1], F32, tag="h16f")
            nc.scalar.activation(h16f[:], phf[0:48, 0:1], AF.Prelu,
                                 bias=b16[:, 1:2], scale=1.0 / 2048.0,
                                 alpha=0.2)
            for w_, (roff, ooff) in enumerate([(0, 0), (32, 4)]):
                for blk in range(4):
                    ph2 = psC.tile([128, 512], F32, tag="pc")
                    nc.tensor.matmul(ph2[:, 0:1],
                                     fcf24[roff:roff + 16,
                                           w_ * 512 + blk * 128:
                                           w_ * 512 + (blk + 1) * 128],
                                     h16f[roff:roff + 16, :],
                                     start=True, stop=True)
                    tt = sb1.tile([128, 1], F32, tag="seftmp")
                    nc.scalar.activation(tt[:], ph2[:, 0:1], AF.Tanh,
                                         bias=bf[:, ooff + blk:ooff + blk + 1],
                                         scale=0.5)
                    nc.vector.tensor_scalar(sef_cols[:, ooff + blk:ooff + blk + 1],
                                            tt[:], 0.5, 0.5, ALU.mult, ALU.add)

            # ---- freq gates m1f/m2f (tanh) + in-place gating ----
            mfrow = sb1.tile([2, 256], F32, tag="mfrow")
            m2fr = sb1.tile([1, 256], F32, tag="m2fr")
            pgf = psC.tile([128, 512], F32, tag="pc")
            for k in range(8):
                fsrc = F1 if k < 4 else F2
                nc.tensor.matmul(pgf[0:2, 0:256], gfre[:, 2 * k:2 * k + 2],
                                 fsrc[:, k % 4, :],
                                 start=(k == 0), stop=(k == 7))
            nc.scalar.activation(R(mfrow[0:2, :]), pgf[0:2, 0:256],
                                 AF.Tanh, scale=0.0625)
            nc.vector.tensor_scalar(R(mfrow[:]), R(mfrow[:]), 0.5, 0.5,
                                    ALU.mult, ALU.add)
            nc.sync.dma_start(R(m2fr[:]), R(mfrow[1:2, :]))
            for (gi, Ff, ooff) in [(0, F1, 4), (1, F2, 0)]:
                mfsrc = mfrow if gi == 0 else m2fr
                pbf = psA.tile([128, 512], F32, tag="pa")
                nc.tensor.matmul(pbf[:, 0:256], R(ones1[:]),
                                 R(mfsrc[0:1, 0:256]), start=True, stop=True)
                for blk in range(4):
                    nc.vector.scalar_tensor_tensor(
                        Ff[:, blk, :], Ff[:, blk, :],
                        sef_cols[:, ooff + blk:ooff + blk + 1], pbf[:, 0:256],
                        ALU.mult, ALU.mult)

            # ---- IWT (no /2) straight into conv pad group 1 (GPS) ----
            for (Ff, cp) in [(F1, cp1), (F2, cp2)]:
                uu = sb1.tile([128, 16, 16], F32, tag="wt0")
                vv = sb1.tile([128, 16, 16], F32, tag="wt1")
                ww = sb1.tile([128, 16, 16], F32, tag="wt2")
                zz = sb1.tile([128, 16, 16], F32, tag="wt3")
                x1 = Ff[:, 0, :].rearrange("p (h w) -> p h w", h=16)
                x2 = Ff[:, 1, :].rearrange("p (h w) -> p h w", h=16)
                x3 = Ff[:, 2, :].rearrange("p (h w) -> p h w", h=16)
                x4 = Ff[:, 3, :].rearrange("p (h w) -> p h w", h=16)
                nc.gpsimd.tensor_sub(uu[:], x1, x2)
                nc.gpsimd.tensor_sub(vv[:], x3, x4)
                nc.gpsimd.tensor_add(ww[:], x1, x2)
                nc.gpsimd.tensor_add(zz[:], x3, x4)
                ov = cp[:, 1, :].rearrange("p (r c) -> p r c", r=34)[
                    :, 1:33, 1:33].rearrange("p (h a) (w b) -> p a b h w",
                                             a=2, b=2)
                nc.vector.tensor_sub(ov[:, 0, 0], uu[:], vv[:])
                nc.vector.tensor_add(ov[:, 1, 0], uu[:], vv[:])
                nc.vector.tensor_sub(ov[:, 0, 1], ww[:], zz[:])
                nc.vector.tensor_add(ov[:, 1, 1], ww[:], zz[:])

        # ================= BN allreduce: split start/finish ===============
        NTOT = float(N_CORES * BL * 1024)
        scl = sb1.tile([128, 2], F32, tag="scl")
        bia = sb1.tile([128, 2], F32, tag="bia")
        ccalls = {}

        def bn_start(j):
            ccin = sb1.tile([128, 2], F32, tag=f"ccin{j}")
            nc.vector.tensor_reduce(ccin[:, 0:1],
                                    stS[:, j * 8:(j + 1) * 8],
                                    mybir.AxisListType.X, ALU.add)
            nc.vector.tensor_reduce(ccin[:, 1:2],
                                    stQ[:, j * 8:(j + 1) * 8],
                                    mybir.AxisListType.X, ALU.add)
            cbi = dram.tile([128, 2], F32, name=f"cbi{j}")
            cbo = dram.tile([128, 2], F32, name=f"cbo{j}")
            nc.sync.dma_start(cbi[:], ccin[:])
            nc.gpsimd.collective_compute(
                "AllReduce", ALU.add, replica_groups=[list(range(N_CORES))],
                ins=[cbi[:].opt()], outs=[cbo[:].opt()])
            ccall = sb1.tile([128, 2], F32, tag=f"ccall{j}")
            nc.sync.dma_start(ccall[:], cbo[:])
            ccalls[j] = ccall

        def bn_finish(j):
            ccall = ccalls[j]
            mt = sb1.tile([128, 1], F32, tag=f"mt{j}")
            qt = sb1.tile([128, 1], F32, tag=f"qt{j}")
            nc.vector.tensor_scalar_mul(mt[:], ccall[:, 0:1], 1.0 / NTOT)
            nc.vector.tensor_scalar_mul(qt[:], ccall[:, 1:2], 1.0 / NTOT)
            var = sb1.tile([128, 1], F32, tag=f"var{j}")
            nc.vector.scalar_tensor_tensor(var[:], mt[:], 1.0, mt[:],
                                           ALU.mult, ALU.mult)
            nc.vector.tensor_sub(var[:], qt[:], var[:])
            nc.vector.tensor_scalar_add(var[:], var[:], 1e-5)
            sd = sb1.tile([128, 1], F32, tag=f"sd{j}")
            nc.scalar.activation(sd[:], var[:], AF.Sqrt)
            rstd = sb1.tile([128, 1], F32, tag=f"rstd{j}")
            nc.vector.reciprocal(rstd[:], sd[:])
            sc0 = sb1.tile([128, 1], F32, tag=f"sc0{j}")
            nc.vector.tensor_mul(sc0[:], bng[:, j:j + 1], rstd[:])
            nc.vector.tensor_scalar_mul(scl[:, j:j + 1], sc0[:], 1.0 / 16.0)
            tb = sb1.tile([128, 1], F32, tag=f"tb{j}")
            nc.vector.tensor_mul(tb[:], mt[:], sc0[:])
            nc.vector.tensor_sub(tb[:], bnb[:, j:j + 1], tb[:])
            nc.vector.tensor_scalar_mul(bia[:, j:j + 1], tb[:], 1.0 / 16.0)

        # ================= interleaved block: conv(i) x kloop(i+1) ========
        # Software-pipelined: two A-steps (exp blocks) of kloop(i+1) are
        # emitted BEFORE phase(i) so PE/ACT have dense work covering
        # phase's serial chain; then Q(k) / A(k+2) / conv interleave.
        def block(i):
            ev = conv_events(i) if i >= 0 else iter(())
            nmm = 108 if i >= 0 else 0
            have_k = (i + 1 >= 0) and (i + 1 < BL)

            def emit_conv(n):
                done = 0
                while done < n:
                    try:
                        e = next(ev)
                    except StopIteration:
                        return
                    if e[0] == 'mm':
                        e[1]()
                        done += 1
                    else:
                        conv_stats(i, e[1], e[2])
                        if i == BL - 1:
                            # last image: kick the BN AllReduce for conv j
                            # right after its stats, overlapping the next
                            # conv / finalize work.
                            bn_start(e[1])

            ebs = {}
            if have_k:
                kloop_begin(i + 1)
                for k in (0, 1):
                    ebs[(k, 0)] = kloop_A(i + 1, k, 0)
                    ebs[(k, 1)] = kloop_A(i + 1, k, 1)
            if i >= 0:
                phase(i)
                if i + 2 < BL:
                    prep_dma(i + 2)
                    prep_gps(i + 2)
            if have_k:
                per_seg = max(1, nmm // 12) if nmm else 0
                for k in range(8):
                    kloop_Q(i + 1, k, 0, ebs.pop((k, 0)))
                    kloop_Q(i + 1, k, 1, ebs.pop((k, 1)))
                    if k + 2 < 8:
                        ebs[(k + 2, 0)] = kloop_A(i + 1, k + 2, 0)
                        emit_conv(per_seg)
                        ebs[(k + 2, 1)] = kloop_A(i + 1, k + 2, 1)
                        emit_conv(per_seg)
            # drain remaining conv events
            emit_conv(10 ** 9)
            if i + 2 < BL:
                prep_pe(i + 2)

        # ================= emission =================
        prep_gps(0)
        prep_pe(0)
        prep_gps(1)
        block(-1)          # kloop(0) solo + prep_pe(1)
        for i in range(BL):
            block(i)       # phase(i) runs inside, covered by A-prefetch

        # ================= finalize: BN + lrelu + 2x bilinear upsample ====
        def finalize_ij(i, j, od):
            yv = yst[:, j * BL + i, :]                       # [128,1024] bf16
            # z = prelu(scl*y + bia) into padded 34-col layout (pads zero)
            zp = zps[(j * BL + i) % 2]
            nc.scalar.activation(zp[:, :, 1:33],
                                 yv.rearrange("p (h w) -> p h w", w=32),
                                 AF.Prelu, scale=scl[:, j:j + 1],
                                 bias=bia[:, j:j + 1], alpha=0.2)
            # column upsample on PE: out[h, 2w+b] = 3*z[h,w] + z[h, w-1+2b]
            # (per col-parity: contiguous rhs, parity-strided PSUM dst;
            #  one PSUM bank per MM -- matmul output cannot cross banks;
            #  zero pads + one fix-MM supply the edge clamp terms)
            # ug = u with one clamp-guard row at head and tail
            ug = fin.tile([128, 2176], BF16, tag="ug")
            for q in range(4):
                pw = psQ.tile([128, 512], F32,
                              tag=("q1" if q % 2 == 0 else "q2"),
                              name=f"pw_{i}_{j}_{q}")
                pwq = pw[:].rearrange("p (h w b) -> p h w b", w=32, b=2)
                zq = zp[:, 8 * q:8 * q + 8, :]
                for b in range(2):
                    nc.tensor.matmul(pwq[:, :, :, b], ident3b[:],
                                     zq[:, :, 1:33], start=(b == 0),
                                     stop=False, skip_group_check=True)
                for b in range(2):
                    nc.tensor.matmul(pwq[:, :, :, b], identb[:],
                                     zq[:, :, 2 * b:2 * b + 32],
                                     start=False, stop=False,
                                     skip_group_check=True)
                zqb = zq
                frhs = AP(zqb.tensor, zqb.offset + 1,
                          [list(zqb.ap[0]), [34, 8], [31, 2]])
                pwb = pw[:]
                fdst = AP(pwb.tensor, pwb.offset,
                          [list(pwb.ap[0]), [64, 8], [63, 2]])
                nc.tensor.matmul(fdst, identb[:], frhs,
                                 start=False, stop=True,
                                 skip_group_check=True)
                nc.vector.tensor_copy(ug[:, 64 + 512 * q:64 + 512 * (q + 1)],
                                      pw[:])
            nc.vector.tensor_copy(ug[:, 0:64], ug[:, 64:128])
            nc.vector.tensor_copy(ug[:, 2112:2176], ug[:, 2048:2112])
            # row upsample: pE[h] = 3u[h]+u[h-1], pO[h] = 3u[h]+u[h+1]
            t3u = fin.tile([128, 2048], BF16, tag="t3u")
            nc.vector.tensor_scalar_mul(t3u[:], ug[:, 64:2112], 3.0)
            pE = fin.tile([128, 2048], BF16, tag="pE")
            pO = fin.tile([128, 2048], BF16, tag="pO")
            nc.vector.tensor_add(pE[:], t3u[:], ug[:, 0:2048])
            nc.vector.tensor_add(pO[:], t3u[:], ug[:, 128:2176])
            # store: even rows from pE, odd rows from pO
            ov = od.ap()[i].rearrange("p (h a w) -> p h a w", a=2, w=64)
            nc.gpsimd.dma_start(ov[:, :, 0, :],
                                pE[:].rearrange("p (h w) -> p h w", w=64))
            nc.sync.dma_start(ov[:, :, 1, :],
                              pO[:].rearrange("p (h w) -> p h w", w=64))

        bn_finish(0)
        for i in range(BL):
            finalize_ij(i, 0, o1_d)
        bn_finish(1)
        for i in range(BL):
            finalize_ij(i, 1, o2_d)

    nc.compile()
    return nc


_NC_CACHE = None


def _get_nc():
    global _NC_CACHE
    if _NC_CACHE is None:
        _NC_CACHE = _build()
    return _NC_CACHE


def _prep_weights(inp):
    import ml_dtypes
    bf = ml_dtypes.bfloat16
    g = lambda k: np.ascontiguousarray(np.asarray(inp[k], dtype=np.float32))
    W = {}
    W["wlinT"] = np.ascontiguousarray(g("W_lin").T).astype(bf)
    W["identb"] = np.eye(128, dtype=np.float32).astype(bf)
    W["ident3b"] = (3.0 * np.eye(128, dtype=np.float32)).astype(bf)
    W["identf"] = np.eye(128, dtype=np.float32)
    W["ones1"] = np.ones((1, 128), np.float32)
    cw = np.zeros((2, 128, 27, 128), np.float32)   # (conv, ci, k=(g,dy,dx), o)
    scales = [0.25, 1.0 / 16.0, 0.25]
    for j, name in enumerate(["conv1_w", "conv2_w"]):
        w = g(name)  # [o, 384, 3, 3]
        for gg in range(3):
            blk = w[:, gg * 128:(gg + 1) * 128].transpose(2, 3, 1, 0) * scales[gg]
            cw[j, :, gg * 9:(gg + 1) * 9, :] = blk.reshape(9, 128, 128).transpose(
                1, 0, 2)
    W["cw"] = cw.astype(bf)
    W["gspa"] = np.stack([g("gate1_w")[:128], g("gate2_w")[:128],
                          g("gate1_w")[128:], g("gate2_w")[128:]], axis=1)
    gf = np.zeros((128, 16), np.float32)
    gf[:, 0::2] = g("gate1f_w").reshape(8, 128).T
    gf[:, 1::2] = g("gate2f_w").reshape(8, 128).T
    W["gfre"] = gf.astype(bf)
    fc13 = np.zeros((128, 2, 48), np.float32)
    fc13[:, :, 0:16] = g("fc1_w").T.reshape(2, 128, 16).transpose(1, 0, 2)
    fc13[:, :, 32:48] = g("fc3_w").T.reshape(2, 128, 16).transpose(1, 0, 2)
    W["fc13T"] = fc13
    fc24 = np.zeros((48, 256), np.float32)
    fc24[0:16, 0:128] = g("fc2_w").T
    fc24[32:48, 128:256] = g("fc4_w").T
    W["fc24T"] = fc24
    fcf13 = np.zeros((128, 8, 48), np.float32)
    fcf13[:, :, 0:16] = g("fc1f_w").T.reshape(8, 128, 16).transpose(1, 0, 2)
    fcf13[:, :, 32:48] = g("fc3f_w").T.reshape(8, 128, 16).transpose(1, 0, 2)
    W["fcf13T"] = fcf13
    fcf24 = np.zeros((48, 1024), np.float32)
    fcf24[0:16, 0:512] = g("fc2f_w").T
    fcf24[32:48, 512:1024] = g("fc4f_w").T
    W["fcf24T"] = fcf24
    b16 = np.zeros((48, 2), np.float32)
    b16[0:16, 0] = g("fc1_b"); b16[32:48, 0] = g("fc3_b")
    b16[0:16, 1] = g("fc1f_b"); b16[32:48, 1] = g("fc3f_b")
    W["bias16"] = b16
    # pre-halved for the tanh-sigmoid form: sigma(z+b) = .5 + .5 tanh(.5 z + .5 b)
    W["bias128h"] = 0.5 * np.stack([g("fc2_b"), g("fc4_b")], axis=1)
    W["biasfh"] = 0.5 * np.concatenate([g("fc2f_b").reshape(4, 128).T,
                                        g("fc4f_b").reshape(4, 128).T], axis=1)
    W["bng"] = np.stack([g("bn1_g"), g("bn2_g")], axis=1)
    W["bnb"] = np.stack([g("bn1_b"), g("bn2_b")], axis=1)
    out = {}
    for k, v in W.items():
        out[k] = np.ascontiguousarray(v)
    return out


def run(inputs, trace=False):
    import ml_dtypes
    bf = ml_dtypes.bfloat16
    nc = _get_nc()
    W = _prep_weights(inputs)
    f1 = np.ascontiguousarray(
        np.asarray(inputs["frame1"], np.float32)).reshape(
        32, 128, 4096).astype(bf)
    f2 = np.ascontiguousarray(
        np.asarray(inputs["frame2"], np.float32)).reshape(
        32, 128, 4096).astype(bf)
    in_maps = []
    for c in range(N_CORES):
        m = dict(W)
        m["frame1"] = np.ascontiguousarray(f1[c * BL:(c + 1) * BL])
        m["frame2"] = np.ascontiguousarray(f2[c * BL:(c + 1) * BL])
        in_maps.append(m)
    res = run_bass_kernel_spmd(nc, in_maps, core_ids=list(range(N_CORES)),
                               trace=trace)
    o1 = np.concatenate([np.asarray(res.results[c]["o1"], dtype=np.float32)
                         for c in range(N_CORES)], axis=0)
    o2 = np.concatenate([np.asarray(res.results[c]["o2"], dtype=np.float32)
                         for c in range(N_CORES)], axis=0)
    return (o1.reshape(32, 128, 64, 64), o2.reshape(32, 128, 64, 64)), res


def kernel(**inputs):
    (o1, o2), _ = run(inputs, trace=False)
    return o1, o2
